# revision 8
# baseline (speedup 1.0000x reference)
"""Trainium2 Bass kernel for NonLocalAttention (fused 1x1 convs + spatial softmax attention).

Reference computation (N=2, C=64, FC=64, CR=32, H=W=96, HW=9216):
    q = relu(wq @ x + bq)          [N, 32, HW]
    k = relu(wk @ fm + bk)         [N, 32, HW]
    v = relu(wa @ fm + ba)         [N, 64, HW]
    s = softmax(q^T k, axis=keys)  [N, HW, HW]
    o = s @ v^T                    [N, HW, 64]
    out = relu(wo @ [x; o^T] + bo) [N, 64, HW]

Sharding: 8 cores = batch(2) x query-rows(4).  Each core handles 2304 query
pixels of one batch element and needs the full fusionmap of that batch.

Per-core kernel (flash-style, score never goes to HBM):
  - score is computed TRANSPOSED: st[key, q] = k^T q via row-packed (K=32)
    matmuls, 3 key-tiles of 128 at a time into 3 PSUM banks.
  - exp is SPLIT between ScalarE (exact LUT exp -> bf16) and VectorE (bf16
    Schraudolph: st16 = int16(s*128*log2e + B), bitcast int16 -> bf16 gives
    2^x with linear mantissa interp, ~3% wiggle that softmax normalization
    largely cancels).  The per-step bank split alternates so both engines
    stay ~equally loaded; scores are >= 0 (q,k relu'd) and <= ~6.6.
  - second matmul contracts keys with lhsT = [v^T | 1] so PSUM row 64
    accumulates the softmax denominator for free.
  - normalize: denominator row spread to [64, W] via DMA, DVE reciprocal
    (parallel lanes, ~100x faster than on 1 partition), gathered back,
    broadcast by a K=1 matmul; then the output 1x1 conv (wo), relu, DMA out.
"""

import sys

sys.path.insert(0, "/opt/trn_rl_repo")

from contextlib import ExitStack

import ml_dtypes
import numpy as np

import concourse.bacc as bacc
import concourse.bass as bass
import concourse.tile as tile
from concourse import mybir
from concourse import bass_utils

C = 64
FC = 64
CR = 32
N = 2
H = W = 96
HW = H * W            # 9216
NCORES = 8
QPC = HW // 4         # queries per core = 2304
NKT = HW // 128       # 72 key tiles
G = 3                 # row-packing group (3 key tiles concurrently)
NJ = NKT // G         # 24 key-tile groups
QCHUNKS = [(0, 512), (512, 512), (1024, 512), (1536, 512), (2048, 256)]

F32 = mybir.dt.float32
F32R = mybir.dt.float32r
BF16 = mybir.dt.bfloat16
I16 = mybir.dt.int16
ATT = BF16

# bf16 Schraudolph constants: y = int16(x * 128*log2e + B16C); the int16 bit
# pattern read as bf16 is ~exp(x) with +-~3% sawtooth (HW-calibrated bias).
LOG2E = 1.4426950408889634
S16 = 128.0 * LOG2E
B16C = 16248.6


def scalar_banks(qi, j):
    """How many of the G score banks ScalarE exp's this step (VectorE takes
    the rest via Schraudolph).  Chunk 0 is Scalar-heavy because the DVE is
    busy with conv relus there."""
    if qi == 0:
        return 3 if j % 2 == 0 else 2
    return 1 if j % 3 == 2 else 2


def build_bass():
    nc = bacc.Bacc(
        "TRN2", target_bir_lowering=False, debug=False, num_devices=NCORES
    )

    x_aug = nc.dram_tensor("x_aug", [C + 1, QPC], F32R, kind="ExternalInput")
    fm_aug = nc.dram_tensor("fm_aug", [FC + 1, HW], BF16, kind="ExternalInput")
    wq_aug = nc.dram_tensor("wq_aug", [C + 1, CR], F32R, kind="ExternalInput")
    wk_aug = nc.dram_tensor("wk_aug", [FC + 1, CR], BF16, kind="ExternalInput")
    wa_aug = nc.dram_tensor("wa_aug", [FC + 1, C + 1], BF16, kind="ExternalInput")
    wox_aug = nc.dram_tensor("wox_aug", [C + 1, C], F32R, kind="ExternalInput")
    woa_t = nc.dram_tensor("woa_t", [C, C], F32R, kind="ExternalInput")
    out_d = nc.dram_tensor("out_c", [C, QPC], F32, kind="ExternalOutput")

    with tile.TileContext(nc) as tc, ExitStack() as ctx:
        consts = ctx.enter_context(tc.tile_pool(name="consts", bufs=1))
        stp = ctx.enter_context(tc.tile_pool(name="stp", bufs=3))
        wk_pool = ctx.enter_context(tc.tile_pool(name="work", bufs=3))
        # PSUM: 2x3 banks score tiles + 2x1 bank shared acc/fin/v-conv = 8
        psA = ctx.enter_context(tc.tile_pool(name="psA", bufs=2, space="PSUM"))
        psO = ctx.enter_context(tc.tile_pool(name="psO", bufs=2, space="PSUM"))

        # ---- constants / inputs in SBUF ----
        NQT = 4
        HWQ = HW // NQT  # 2304 = 18 key tiles per quarter
        FMq = [
            consts.tile([FC + 1, HWQ], BF16, tag=f"fm{p}", name=f"FM{p}")
            for p in range(NQT)
        ]
        XA = consts.tile([C + 1, QPC], F32R)         # x chunk + ones row
        WQ = consts.tile([C + 1, CR], F32R)
        WK = consts.tile([FC + 1, CR], BF16)
        WA = consts.tile([FC + 1, C + 1], BF16)
        WOX = consts.tile([C + 1, C], F32R)
        WOA = consts.tile([C, C], F32R)
        # DMA order = critical path order: k-conv q0 needs WK+FM0, q-conv
        # chunk 0 needs WQ+XA.
        nc.sync.dma_start(WK[:], wk_aug.ap())
        nc.sync.dma_start(FMq[0][:], fm_aug.ap()[:, 0:HWQ])
        nc.sync.dma_start(WQ[:], wq_aug.ap())
        nc.sync.dma_start(XA[:], x_aug.ap())
        for p in range(1, NQT):
            nc.sync.dma_start(FMq[p][:], fm_aug.ap()[:, p * HWQ : (p + 1) * HWQ])
        nc.sync.dma_start(WA[:], wa_aug.ap())
        nc.sync.dma_start(WOX[:], wox_aug.ap())
        nc.sync.dma_start(WOA[:], woa_t.ap())

        def fm_kt(kt):  # [65, 128] slice of fusionmap for key tile kt
            p, i = divmod(kt, 18)
            return FMq[p][:, 128 * i : 128 * (i + 1)]

        # KR: k channels row-packed: partitions 32g..32g+31 hold key tile
        # kt=3j+g at free block j; per-quarter for earlier start.
        # QR: per-chunk tiles, q replicated on partition groups 0..2.
        KRq = [
            consts.tile([128, NJ // NQT, 128], ATT, tag=f"kr{p}", name=f"KR{p}")
            for p in range(NQT)
        ]
        QRc = [
            consts.tile([128, qn], ATT, tag=f"qr{ci}", name=f"QR{ci}")
            for ci, (q0, qn) in enumerate(QCHUNKS)
        ]
        # VT: [keys(128), kt, C+1]; column C is 1.0 straight out of the
        # v-conv (wa is augmented with a ones column), so mm2 accumulates the
        # softmax denominator in PSUM row C for free.
        NVR = NKT // 4
        VTr = [
            consts.tile([128, 4, C + 1], ATT, tag=f"vt{r}", name=f"VT{r}")
            for r in range(NVR)
        ]
        ones1 = consts.tile([1, C], F32R)
        nc.vector.memset(ones1[:].bitcast(F32), 1.0)

        # Preload the exp table set (~2.7us) off the critical path, before
        # the first real exp.
        dummy = wk_pool.tile([1, 1], F32, tag="dummy", name="dummy")
        nc.scalar.activation(
            dummy[:], ones1[0:1, 0:1].bitcast(F32),
            mybir.ActivationFunctionType.Exp,
        )

        # ---- phase 1: q / k convs ----
        # k quarter p: plain [32, HW/4] as 6 x 384-wide chunks, then
        # interleave kt%3 -> partition group via DMA
        Ksq = [
            consts.tile([CR, HWQ], ATT, tag=f"ks{p}", name=f"Ks{p}")
            for p in range(NQT)
        ]

        def k_chunk(p, c):
            ps = psA.tile([128, G, 512], F32, tag="sc", name="kps")
            nc.tensor.matmul(
                ps[0:CR, 0, 0:384], WK[:], FMq[p][:, 384 * c : 384 * (c + 1)]
            )
            nc.vector.tensor_scalar_max(
                Ksq[p][:, 384 * c : 384 * (c + 1)], ps[0:CR, 0, 0:384], 0.0
            )
            if c == HWQ // 384 - 1:
                Ksv = Ksq[p].rearrange("p (j g c) -> p j g c", g=G, c=128)
                for g in range(G):
                    nc.sync.dma_start(
                        KRq[p][32 * g : 32 * g + 32, :, :], Ksv[:, :, g, :]
                    )

        # q chunk ci: relu into QRc[ci][0:32], then replicate to groups 1, 2
        def q_chunk(ci):
            q0, qn = QCHUNKS[ci]
            ps = psA.tile([128, G, 512], F32, tag="sc", name="qps")
            nc.tensor.matmul(
                ps[0:CR, 0, 0:qn], WQ[:], XA[:, q0 : q0 + qn]
            )
            nc.vector.tensor_scalar_max(
                QRc[ci][0:CR, 0:qn], ps[0:CR, 0, 0:qn], 0.0
            )
            nc.sync.dma_start(QRc[ci][32:64, 0:qn], QRc[ci][0:32, 0:qn])
            nc.sync.dma_start(QRc[ci][64:96, 0:qn], QRc[ci][0:32, 0:qn])

        # prologue: only what the first attention step needs (quarter 0 of
        # the k conv and query chunk 0); the rest is emitted inside chunk
        # 0's loop so the PE, DVE and ScalarE all saturate from the start.
        for c in range(HWQ // 384):
            k_chunk(0, c)
        q_chunk(0)

        # v^T conv round r: out[key, 0:64] = relu'd v, out[key, 64:128] = 1.0
        def v_round(r):
            ps = psO.tile([128, 512], F32, tag="acc")
            for i in range(4):
                kt = 4 * r + i
                nc.tensor.matmul(
                    ps[:, (C + 1) * i : (C + 1) * (i + 1)], fm_kt(kt), WA[:]
                )
            nc.vector.tensor_scalar_max(
                VTr[r][:], ps[:, 0 : 4 * (C + 1)], 0.0
            )

        # ---- phase 2: attention + output conv, per query chunk ----
        # The normalize + output conv of chunk qc is emitted inside chunk
        # qc+1's loop, in two halves: the reciprocal chain early (j==1) so
        # its DMA/DVE latency is long gone when the PE-side half (j==4)
        # reaches the in-order PE queue (a stalled PE > 3.4us re-throttles
        # the HAM clock gate to half rate).
        def finalize_a(acc, q0, qn):
            # row C of acc = sum_k exp(score).  Spread the [1, qn] row over
            # 64 partitions by DMA so the DVE reciprocal runs on parallel
            # lanes (~100x faster than on 1 partition), gather back.
            Wd = qn // 64
            DS = wk_pool.tile([1, 512], F32, tag="ds", name="ds")
            nc.scalar.copy(DS[:, 0:qn], acc[C : C + 1, 0:qn])
            DD = wk_pool.tile([64, 8], F32, tag="dd", name="dd")
            nc.sync.dma_start(DD[:, 0:Wd], DS[:, 0:qn])
            RR = wk_pool.tile([64, 8], F32, tag="rr", name="rr")
            with nc.allow_low_precision(reason="softmax denom reciprocal"):
                nc.vector.reciprocal(RR[:, 0:Wd], DD[:, 0:Wd])
            RCP = wk_pool.tile([1, 512], F32R, tag="rc", name="rc")
            nc.sync.dma_start(RCP[:, 0:qn].bitcast(F32), RR[:, 0:Wd])
            return RCP

        def finalize_b(acc, q0, qn, RCP):
            # Broadcast 1/denom to 64 partitions with a K=1 matmul.
            rb_ps = psA.tile([128, G, 512], F32, tag="sc", name="rb_ps")
            nc.tensor.matmul(rb_ps[0:C, 0, 0:qn], ones1[:], RCP[:, 0:qn])
            rbS = wk_pool.tile([C, 512], F32, tag="rbS", name="rbS")
            nc.scalar.copy(rbS[:, 0:qn], rb_ps[0:C, 0, 0:qn])
            return rbS

        def finalize_c(acc, q0, qn, rbS):
            attn = wk_pool.tile([C, 512], F32R, tag="attn", name="attn")
            nc.vector.tensor_mul(attn[:, 0:qn], acc[0:C, 0:qn], rbS[:, 0:qn])
            # out = relu(wo_x @ x + wo_a @ attn + bo); fin is allocated only
            # after the attn mul (acc's last reader) so the psO slot reuse
            # of acc(qi-1) cannot cycle.
            fin = psO.tile([128, 512], F32, tag="acc", name="fin")
            nc.tensor.matmul(
                fin[0:C, 0:qn], WOX[:], XA[:, q0 : q0 + qn],
                start=True, stop=False,
            )
            nc.tensor.matmul(
                fin[0:C, 0:qn], WOA[:], attn[:, 0:qn],
                start=False, stop=True,
            )
            outs = wk_pool.tile([C, 512], F32, tag="outs", name="outs")
            nc.vector.tensor_scalar_max(outs[:, 0:qn], fin[0:C, 0:qn], 0.0)
            nc.sync.dma_start(out_d.ap()[:, q0 : q0 + qn], outs[:, 0:qn])

        # Flat software-pipelined emission over t = (chunk, j): the PE FIFO
        # sees [mm1(t), mm2(t-1), mm1(t+1), ...] with no boundary structure,
        # so chunk transitions cost nothing.  mm2 payloads lag one step.
        NT = len(QCHUNKS) * NJ
        accs = [None] * len(QCHUNKS)
        pend_mm2 = None
        pend_fin = None
        pend_rcp = None
        vr_next = 0

        def emit_mm2(qi, j, srcs):
            q0, qn = QCHUNKS[qi]
            for g in range(G):
                kt = G * j + g
                nc.tensor.matmul(
                    accs[qi][0 : C + 1, 0:qn],
                    VTr[kt // 4][:, kt % 4, :],
                    srcs[g],
                    start=(j == 0 and g == 0),
                    stop=(j == NJ - 1 and g == G - 1),
                )

        for t in range(NT):
            qi, j = divmod(t, NJ)
            q0, qn = QCHUNKS[qi]
            if j == 0:
                accs[qi] = psO.tile([128, 512], F32, tag="acc", name="acc")
            if qi == 0:
                # remaining k-conv quarters, front-loaded so each quarter's
                # interleave DMA lands well before its first mm1 consumer
                # (quarter p needed from j = 6p)
                KSCHED = [3, 3, 2, 2, 2, 2, 2, 2]
                if j < len(KSCHED):
                    base = 6 + sum(KSCHED[:j])
                    for kc in range(base, base + KSCHED[j]):
                        k_chunk(kc // 6, kc % 6)
                # remaining q-conv chunks, well before their chunk starts
                if j in (2, 4, 6, 8):
                    q_chunk(j // 2)
            # keep v-conv a little ahead of the mm2 consumer on chunk 0
            while vr_next < NVR and (qi > 0 or 4 * vr_next <= 3 * j + 6):
                v_round(vr_next)
                vr_next += 1
            sc = psA.tile([128, G, 512], F32, tag="sc")
            jq, jj = divmod(j, NJ // NQT)
            for g in range(G):
                nc.tensor.matmul(
                    sc[:, g, 0:qn],
                    KRq[jq][32 * g : 32 * g + 32, jj, :],
                    QRc[qi][32 * g : 32 * g + 32, 0:qn],
                )
            # split exp: ScalarE takes banks [0, gs) exactly into st_s, the
            # DVE Schraudolphs banks [gs, G) into its own int16 tile (a
            # shared tile would serialize the engines via false W-W deps).
            gs = scalar_banks(qi, j)
            st_s = stp.tile([128, G, 512], ATT, tag="st")
            nc.scalar.activation(
                st_s[:, 0:gs, 0:qn],
                sc[:, 0:gs, 0:qn],
                mybir.ActivationFunctionType.Exp,
            )
            srcs = [st_s[:, g, 0:qn] for g in range(gs)]
            if gs < G:
                st_v = stp.tile([128, G - 1, 512], I16, tag="stv")
                nc.vector.tensor_scalar(
                    st_v[:, 0 : G - gs, 0:qn],
                    sc[:, gs:G, 0:qn],
                    S16,
                    B16C,
                    mybir.AluOpType.mult,
                    mybir.AluOpType.add,
                )
                srcs += [
                    st_v.bitcast(ATT)[:, g, 0:qn] for g in range(G - gs)
                ]
            if pend_mm2 is not None:
                emit_mm2(*pend_mm2)
            pend_mm2 = (qi, j, srcs)
            if pend_fin is not None:
                if j == 1:
                    pend_rcp = finalize_a(*pend_fin)
                elif j == 3:
                    pend_rcp = finalize_b(*pend_fin, pend_rcp)
                elif j == 4:
                    finalize_c(*pend_fin, pend_rcp)
                    pend_fin = None
                    pend_rcp = None
            if j == NJ - 1:
                pend_fin = (accs[qi], q0, qn)
        emit_mm2(*pend_mm2)
        rcp = finalize_a(*pend_fin)
        rbs = finalize_b(*pend_fin, rcp)
        finalize_c(*pend_fin, rbs)

    nc.compile()
    return nc


_NC_CACHE = None


def _get_nc():
    global _NC_CACHE
    if _NC_CACHE is None:
        _NC_CACHE = build_bass()
    return _NC_CACHE


def make_in_maps(x, fusionmap, wq, bq, wk, bk, wa, ba, wo, bo):
    x = np.asarray(x, np.float32)
    fm = np.asarray(fusionmap, np.float32)
    xf = x.reshape(N, C, HW)
    fmf = fm.reshape(N, FC, HW)
    ones_hw = np.ones((1, HW), np.float32)
    wq_aug = np.concatenate(
        [np.asarray(wq).T, np.asarray(bq)[None, :]], 0
    ).astype(np.float32)
    wk_aug = np.concatenate(
        [np.asarray(wk).T, np.asarray(bk)[None, :]], 0
    ).astype(ml_dtypes.bfloat16)
    # [wa^T | 0; ba | 1]: columns C..2C-1 evaluate to exactly 1.0 after the
    # conv (ones row of fm_aug x ones), giving mm2 its denominator columns.
    wa_blk = np.concatenate([np.asarray(wa).T, np.asarray(ba)[None, :]], 0)
    ones_blk = np.concatenate(
        [np.zeros((FC, 1), np.float32), np.ones((1, 1), np.float32)], 0
    )
    wa_aug = np.concatenate([wa_blk, ones_blk], 1).astype(ml_dtypes.bfloat16)
    wo = np.asarray(wo, np.float32)
    wox_aug = np.concatenate(
        [wo[:, :C].T, np.asarray(bo)[None, :]], 0
    ).astype(np.float32)
    woa_t = np.ascontiguousarray(wo[:, C:].T).astype(np.float32)

    in_maps = []
    for core in range(NCORES):
        n, c = divmod(core, 4)
        x_chunk = xf[n][:, c * QPC : (c + 1) * QPC]
        x_aug = np.concatenate([x_chunk, ones_hw[:, :QPC]], 0)
        fm_aug = np.concatenate([fmf[n], ones_hw], 0).astype(ml_dtypes.bfloat16)
        in_maps.append(
            {
                "x_aug": np.ascontiguousarray(x_aug),
                "fm_aug": np.ascontiguousarray(fm_aug),
                "wq_aug": wq_aug,
                "wk_aug": wk_aug,
                "wa_aug": wa_aug,
                "wox_aug": wox_aug,
                "woa_t": woa_t,
            }
        )
    return in_maps


def run(in_maps, trace=False, tmpdir=None):
    nc = _get_nc()
    return bass_utils.run_bass_kernel_spmd(
        nc,
        in_maps,
        core_ids=list(range(NCORES)),
        trace=trace,
        tmpdir=tmpdir,
    )


def kernel(**inputs):
    in_maps = make_in_maps(**inputs)
    res = run(in_maps)
    out = np.empty((N, C, HW), np.float32)
    for core in range(NCORES):
        n, c = divmod(core, 4)
        out[n][:, c * QPC : (c + 1) * QPC] = res.results[core]["out_c"]
    return out.reshape(N, C, H, W)


if __name__ == "__main__":
    import reference

    inputs = {k: np.asarray(v) for k, v in reference.setup_inputs().items()}
    got = kernel(**inputs)
    print("kernel output", got.shape, got.dtype)
